# revision 34
# baseline (speedup 1.0000x reference)
"""Trainium2 Bass kernel for ClassicalReconstructionHydraSSMCore.

Quantum statevector simulation: batch 8192, 10 qubits, three circuits
(forward/backward/diagonal), combine + normalize + Pauli X/Y/Z measure.

Sharding: pure data parallel over batch across 8 cores (1024 each).
Per-core layout: batch on partitions (8 tiles of 128), state on free dim
as fp16 [re(1024) | im(1024)].

v2 design (cost-model driven):
 - rot gates (folded RZ*RY*RX per wire) run on the Tensor engine as
   diagonal-weight matmuls: per-batch scalars become diag(u) 128x128
   weights, terms accumulate in PSUM fp32, then one evict op converts
   back to fp16 SBUF.
 - CRX gates are striped between a 4-op DVE form (two 4x-mode
   tensor_scalar partials + two 2x tensor_tensor combines, partly on
   Pool) and the PE diag-matmul form.
 - Tiles are software-pipelined: rot(t) [PE-heavy] is interleaved with
   rings0(t+1) [DVE/Pool-heavy], rings1(t) with rot(t+1), tail(t) with
   rings0(t+2), so no engine starves during phase transitions.
"""

import numpy as np

import concourse.bass as bass
import concourse.tile as tile
from concourse import bacc, mybir

F32 = mybir.dt.float32
F16 = mybir.dt.float16
AOT = mybir.AluOpType
ACTF = mybir.ActivationFunctionType


def _register_axpby():
    """Runtime-register a custom DVE op: out = in0*s0 + in1*s1."""
    import concourse.dve_ops as dve_ops
    from concourse.dve_spec import Spec, Src0, Src1, C0, C1, lower
    from concourse.dve_spec import _has_src1 as has_src1
    from concourse.dve_uop import DveOpSpec

    name = "AXPBY9_ANT"
    for op in dve_ops.OPS:
        if op.name == name:
            return op
    spec = Spec(
        body=Src0 * C0 + Src1 * C1,
        reference=lambda in0, in1, s0, s1, imm2: in0 * s0 + in1 * s1,
    )
    row = dve_ops._CUSTOM_DVE_ROW_BASE + len(dve_ops.OPS)
    assert row < 0x20
    dve_ops._SUB_OPCODE_FOR_NAME[name] = row
    shas = {}
    for ver in ("v3", "v4"):
        s = DveOpSpec(
            name=name, opcode=row, uops=lower(spec, ver=ver), rd1_en=has_src1(spec)
        )
        shas[ver] = s.sha(ver)
    op = dve_ops.DveOp(name, spec, subdim=False, uops_sha=shas)
    dve_ops.OPS.append(op)
    dve_ops.CUSTOM_DVE_SPECS[name] = spec
    return op


AXPBY = _register_axpby()

NQ = 10
DIM = 1 << NQ          # 1024
HD = DIM // 2          # 512
P = 128
N_CORES = 8
B_CORE = 1024
NT = B_CORE // P       # 8 tiles per core
PI_2 = float(np.pi / 2)

FWD, BWD, DIAG = 0, 1, 2

# param column layout on device (310 cols):
#  rot block [0,180): col(c,L,w,k) = 60c+30L+3w+k, wire-indexed for all
#    circuits (host rearrange absorbs BWD's reversed wire order).
#    cols [0,120) (fwd+bwd) are dt-scaled, [120,180) (diag) are not.
#  crx block [180,300): col(c,L,j) = 180+40c+20L+j, j = time order.
#  angles [300,310).
NCOL = 310
CRX0 = 180
ANG0 = 300


def rot_col(c, L, w, k):
    return 60 * c + 30 * L + 3 * w + k


def crx_col(c, L, j):
    return CRX0 + 40 * c + 20 * L + j


def _ring_gates(c, L):
    """Time-ordered entangler list [(ctrl, tgt, col)] for circuit c, layer L."""
    out = []
    if c in (FWD, DIAG):
        for k in range(NQ):       # ring1: CRX(i, i+1), i ascending
            out.append((k, (k + 1) % NQ, crx_col(c, L, k)))
        for k in range(NQ):       # ring2: CRX(i, i-1), i descending
            i = NQ - 1 - k
            out.append((i, (i - 1) % NQ, crx_col(c, L, NQ + k)))
    else:  # BWD
        for k in range(NQ):       # ring1: CRX(i, i-1), i descending
            i = NQ - 1 - k
            out.append((i, (i - 1) % NQ, crx_col(c, L, k)))
        for k in range(NQ):       # ring2: CRX(i, i+1), i ascending
            out.append((k, (k + 1) % NQ, crx_col(c, L, NQ + k)))
    return out


def _crx_geom(S, ctrl, tgt):
    """Views for a CRX(ctrl,tgt) gate on state tile AP S (P, 2048)."""
    hi, lo = (ctrl, tgt) if ctrl < tgt else (tgt, ctrl)
    if lo - hi == 1:
        a = 1 << hi
        z = 1 << (8 - hi)
        v = S.rearrange("p (pl a x y z) -> p pl a x y z", pl=2, a=a, x=2, y=2, z=z)
        if ctrl == hi:
            q = lambda pl, t: v[:, pl, :, 1, t, :]
            ht = lambda t: v[:, :, :, 1, t, :]
            def half(plrev=False, trev=False):
                h = v[:, :, :, 1, :, :]      # (pl, a, t, z)
                if plrev:
                    h = h[:, ::-1]
                if trev:
                    h = h[:, :, :, ::-1, :]
                return h
        else:
            q = lambda pl, t: v[:, pl, :, t, 1, :]
            ht = lambda t: v[:, :, :, t, 1, :]
            def half(plrev=False, trev=False):
                h = v[:, :, :, :, 1, :]      # (pl, a, t, z)
                if plrev:
                    h = h[:, ::-1]
                if trev:
                    h = h[:, :, :, ::-1, :]
                return h
        tmaj = lambda: half().transpose([0, 3, 1, 2, 4])
        wx = lambda W: W.rearrange("p (pl a t z) -> p pl a t z", pl=2, a=a, t=2, z=z)
        psv = lambda pt: pt.rearrange("p (t pl a z) -> p t pl a z", t=2, pl=2, a=a, z=z)
    else:
        b = DIM // 4
        v = S.rearrange("p (pl x b y) -> p pl x b y", pl=2, x=2, b=b, y=2)
        if ctrl == 0:
            # ctrl-dim = x, tgt-dim = y; half dims (pl, b, t)
            q = lambda pl, t: v[:, pl, 1, :, t]
            ht = lambda t: v[:, :, 1, :, t]
            def half(plrev=False, trev=False):
                h = v[:, :, 1, :, :]         # (pl, b, t)
                if plrev:
                    h = h[:, ::-1]
                if trev:
                    h = h[:, :, :, ::-1]
                return h
            tmaj = lambda: half().transpose([0, 3, 1, 2])
            wx = lambda W: W.rearrange("p (pl b t) -> p pl b t", pl=2, b=b, t=2)
        else:
            # ctrl == NQ-1 (dim y), tgt-dim = x; half dims (pl, t, b)
            q = lambda pl, t: v[:, pl, t, :, 1]
            ht = lambda t: v[:, :, t, :, 1]
            def half(plrev=False, trev=False):
                h = v[:, :, :, :, 1]         # (pl, t, b)
                if plrev:
                    h = h[:, ::-1]
                if trev:
                    h = h[:, :, ::-1, :]
                return h
            tmaj = lambda: half().transpose([0, 2, 1, 3])
            wx = lambda W: W.rearrange("p (pl t b) -> p pl t b", pl=2, t=2, b=b)
        psv = lambda pt: pt.rearrange("p (t pl b) -> p t pl b", t=2, pl=2, b=b)
    return q, ht, half, tmaj, wx, psv


class _Ctx:
    pass


def emit_core_kernel(nc, tc, ins, outs, n_tiles=NT, real_cf=True):
    ps_d = ins["ps"]
    dth_d = ins["dth"]
    cf_d = ins["cf"]
    msk_d = ins["masks"]
    out_d = outs["out"]

    tsd = nc.vector.tensor_scalar_mul
    ttd = nc.vector.tensor_tensor
    ttp = nc.gpsimd.tensor_tensor
    ax = lambda out, x, sx, y, sy: nc.vector._custom_dve(
        AXPBY, out=out, in0=x, in1=y, s0=sx, s1=sy
    )

    # gate->engine striping (tuned via TimelineSim):
    #  ("PE", e): diag-matmul form; e = evict engine 'A'/'D'/'P'
    #  ("DV", e0e1): DVE partials; combines on e0 (re) / e1 (im), 'D'/'P'
    CRX_PATTERN = [
        ("PE", "A"), ("DV", "DP"), ("PE", "A"), ("DV", "DP"),
        ("PE", "A"), ("DV", "DP"), ("DV", "PP"), ("DV", "DP"),
    ]
    crx_ctr = [0]
    ps_ctr = [0]

    with (
        tc.tile_pool(name="const", bufs=1) as cpool,
        tc.tile_pool(name="work", bufs=3) as pool,
        tc.tile_pool(name="state", bufs=4) as spool,
        tc.tile_pool(name="psum", bufs=2, space="PSUM") as pspool,
    ):
        cf_t = cpool.tile([P, 16], F32)
        nc.sync.dma_start(cf_t[:, 0 : cf_d.shape[1]], cf_d[:])
        msk = cpool.tile([P, 384], F16)
        nc.sync.dma_start(msk[:], msk_d[:])
        mask = msk[:, 0:128]       # identity
        maskPM = msk[:, 128:384]   # [I | -I]
        pi2 = cpool.tile([P, 1], F32)
        nc.gpsimd.memset(pi2[:], PI_2)
        pi2c = pi2[:, 0:1]

        def _nfree(ap):
            return len(ap.opt().ap) - 1

        def ts_auto(out_v, in_v, sc):
            """TS, split along the plane dim if >3 free dims after opt."""
            if _nfree(out_v) <= 3 and _nfree(in_v) <= 3:
                tsd(out_v, in_v, sc)
            else:
                for pl in range(2):
                    tsd(out_v[:, pl], in_v[:, pl], sc)

        def mm(out_ap, w_ap, mov_ap, start, stop):
            nc.tensor.matmul(out_ap, w_ap, mov_ap, start=start, stop=stop)

        # ================= prologue =================
        def emit_prologue(t):
            X = _Ctx()
            X.t = t
            r0, r1 = t * P, (t + 1) * P
            X.r0, X.r1 = r0, r1
            ps = pool.tile([P, NCOL], F32, tag="ps")
            nc.sync.dma_start(ps[:], ps_d[r0:r1, :])
            dth = pool.tile([P, 1], F32, tag="dth")
            nc.sync.dma_start(dth[:], dth_d[r0:r1, :])

            sh = pool.tile([P, NCOL], F32, tag="sh")
            ch = pool.tile([P, NCOL], F32, tag="ch")
            trA = pool.tile([P, NCOL], F32, tag="trA")
            trB = pool.tile([P, NCOL], F32, tag="trB")
            nc.scalar.activation(sh[:, 0:120], ps[:, 0:120], ACTF.Sin, scale=dth[:, 0:1])
            nc.scalar.activation(sh[:, 120:NCOL], ps[:, 120:NCOL], ACTF.Sin, scale=0.25)
            nc.scalar.activation(
                ch[:, 0:120], ps[:, 0:120], ACTF.Sin, scale=dth[:, 0:1], bias=pi2c
            )
            nc.scalar.activation(
                ch[:, 120:NCOL], ps[:, 120:NCOL], ACTF.Sin, scale=0.25, bias=pi2c
            )
            ttp(trA[:], sh[:], ch[:], op=AOT.mult)
            ttp(trB[:], sh[:], sh[:], op=AOT.mult)
            nc.gpsimd.tensor_scalar_mul(sh[:], trA[:], 2.0)
            nc.gpsimd.tensor_scalar(ch[:], trB[:], -2.0, 1.0, op0=AOT.mult, op1=AOT.add)
            X.sh, X.ch = sh, ch

            # u-coefficients per layer
            rotc = ch[:, 0:180].rearrange("p (c L w k) -> p c L w k", c=3, L=2, w=10, k=3)
            rots = sh[:, 0:180].rearrange("p (c L w k) -> p c L w k", c=3, L=2, w=10, k=3)
            m1 = pool.tile([P, 30], F32, tag="m1")
            m2 = pool.tile([P, 30], F32, tag="m2")
            m3 = pool.tile([P, 30], F32, tag="m3")
            m4 = pool.tile([P, 30], F32, tag="m4")
            w1 = pool.tile([P, 30], F32, tag="w1")
            w2 = pool.tile([P, 30], F32, tag="w2")
            V = lambda tl: tl[:].rearrange("p (c g) -> p c g", c=3, g=10)
            U = []
            for L in range(2):
                ca = rotc[:, :, L, :, 0]
                cb = rotc[:, :, L, :, 1]
                cg = rotc[:, :, L, :, 2]
                sa = rots[:, :, L, :, 0]
                sb = rots[:, :, L, :, 1]
                sg = rots[:, :, L, :, 2]
                u = {
                    k: pool.tile([P, 30], F32, tag=f"u{k}{L}", name=f"u{k}{L}_{t}")
                    for k in ("p", "q", "nr", "s")
                }
                ttp(V(m1), cb, ca, op=AOT.mult)
                ttp(V(m2), sb, sa, op=AOT.mult)
                ttp(V(m3), sb, ca, op=AOT.mult)
                ttp(V(m4), cb, sa, op=AOT.mult)
                ttp(V(w1), cg, V(m1), op=AOT.mult)
                ttp(V(w2), sg, V(m2), op=AOT.mult)
                ttp(V(u["p"]), V(w1), V(w2), op=AOT.add)
                ttp(V(w1), cg, V(m2), op=AOT.mult)
                ttp(V(w2), sg, V(m1), op=AOT.mult)
                ttp(V(u["q"]), V(w1), V(w2), op=AOT.subtract)
                ttp(V(w1), cg, V(m3), op=AOT.mult)
                ttp(V(w2), sg, V(m4), op=AOT.mult)
                ttp(V(u["nr"]), V(w1), V(w2), op=AOT.add)
                ttp(V(w1), sg, V(m3), op=AOT.mult)
                ttp(V(w2), cg, V(m4), op=AOT.mult)
                ttp(V(u["s"]), V(w1), V(w2), op=AOT.subtract)
                U.append(u)
            X.U = U

            # v vectors: layer-0 rotations folded into init
            u0 = U[0]
            angc = ch[:, ANG0:ANG0 + 10]
            angs = sh[:, ANG0:ANG0 + 10]
            a3c = pool.tile([P, 30], F32, tag="a3c")
            a3s = pool.tile([P, 30], F32, tag="a3s")
            for c in range(3):
                nc.scalar.copy(a3c[:, 10 * c : 10 * c + 10], angc)
                nc.scalar.copy(a3s[:, 10 * c : 10 * c + 10], angs)
            v0r = pool.tile([P, 30], F32, tag="v0r")
            v0i = pool.tile([P, 30], F32, tag="v0i")
            v1r = pool.tile([P, 30], F32, tag="v1r")
            v1i = pool.tile([P, 30], F32, tag="v1i")
            nv0i = pool.tile([P, 30], F32, tag="nv0i")
            nv1i = pool.tile([P, 30], F32, tag="nv1i")
            ttp(w1[:], u0["p"][:], a3c[:], op=AOT.mult)
            ttp(w2[:], u0["nr"][:], a3s[:], op=AOT.mult)
            ttp(v0r[:], w1[:], w2[:], op=AOT.subtract)
            ttp(w1[:], u0["q"][:], a3c[:], op=AOT.mult)
            ttp(w2[:], u0["s"][:], a3s[:], op=AOT.mult)
            ttp(v0i[:], w1[:], w2[:], op=AOT.add)
            ttp(w1[:], u0["nr"][:], a3c[:], op=AOT.mult)
            ttp(w2[:], u0["p"][:], a3s[:], op=AOT.mult)
            ttp(v1r[:], w1[:], w2[:], op=AOT.add)
            ttp(w1[:], u0["s"][:], a3c[:], op=AOT.mult)
            ttp(w2[:], u0["q"][:], a3s[:], op=AOT.mult)
            ttp(v1i[:], w1[:], w2[:], op=AOT.subtract)
            nc.gpsimd.tensor_scalar_mul(nv0i[:], v0i[:], -1.0)
            nc.gpsimd.tensor_scalar_mul(nv1i[:], v1i[:], -1.0)

            # product-state build
            st = [spool.tile([P, 2 * DIM], F16, tag=f"st{c}", name=f"st{c}_{t}") for c in range(3)]
            X.st = st
            ab = [
                [pool.tile([P, 32], F32, tag=f"ab{c}_{k}", name=f"ab{c}_{k}") for k in range(8)]
                for c in range(3)
            ]
            adup = [pool.tile([P, 192], F16, tag=f"adup{c}", name=f"adup{c}") for c in range(3)]
            scr1 = pool.tile([P, DIM], F16, tag="scr1")
            scr2 = pool.tile([P, DIM], F16, tag="scr2")
            X.scr1 = scr1

            def expand(c, bufs, wires):
                br, bi, br2, bi2 = bufs
                j0 = 10 * c + wires[0]
                for dst, src in (
                    (br[:, 0:1], v0r), (br[:, 1:2], v1r),
                    (bi[:, 0:1], v0i), (bi[:, 1:2], v1i),
                ):
                    tsd(dst, src[:, j0 : j0 + 1], 1.0)
                width = 2
                cur_r, cur_i, oth_r, oth_i = br, bi, br2, bi2
                for w in wires[1:]:
                    j = 10 * c + w
                    c0r, c0i = v0r[:, j : j + 1], v0i[:, j : j + 1]
                    c1r, c1i = v1r[:, j : j + 1], v1i[:, j : j + 1]
                    n0i, n1i = nv0i[:, j : j + 1], nv1i[:, j : j + 1]
                    old_r, old_i = cur_r[:, 0:width], cur_i[:, 0:width]
                    nw = 2 * width
                    nr_v = oth_r[:, 0:nw].rearrange("p (w t) -> p w t", w=width, t=2)
                    ni_v = oth_i[:, 0:nw].rearrange("p (w t) -> p w t", w=width, t=2)
                    ax(nr_v[:, :, 0], old_r, c0r, old_i, n0i)
                    ax(ni_v[:, :, 0], old_r, c0i, old_i, c0r)
                    ax(nr_v[:, :, 1], old_r, c1r, old_i, n1i)
                    ax(ni_v[:, :, 1], old_r, c1i, old_i, c1r)
                    cur_r, oth_r = oth_r, cur_r
                    cur_i, oth_i = oth_i, cur_i
                    width = nw
                return cur_r, cur_i

            for c in range(3):
                ar, ai = expand(c, ab[c][0:4], list(range(5)))
                br_, bi_ = expand(c, ab[c][4:8], list(range(5, NQ)))
                ad = adup[c]
                nc.scalar.copy(
                    ad[:, 0:64].rearrange("p (i t) -> p i t", i=32, t=2),
                    ar[:, 0:32].rearrange("p (i t) -> p i t", i=32, t=1).broadcast_to([P, 32, 2]),
                )
                nc.scalar.copy(
                    ad[:, 64:128].rearrange("p (i t) -> p i t", i=32, t=2),
                    ai[:, 0:32].rearrange("p (i t) -> p i t", i=32, t=1).broadcast_to([P, 32, 2]),
                )
                nc.scalar.copy(ad[:, 128:160], br_[:, 0:32])
                nc.scalar.copy(ad[:, 160:192], bi_[:, 0:32])

            for c in range(3):
                ad = adup[c]
                jv = lambda sl: sl.rearrange("p (i o t) -> p i o t", i=32, o=1, t=2).broadcast_to([P, 32, 16, 2])
                bv = lambda sl: sl.rearrange("p (o j t) -> p o j t", o=1, j=16, t=2).broadcast_to([P, 32, 16, 2])
                arv, aiv = jv(ad[:, 0:64]), jv(ad[:, 64:128])
                brv, biv = bv(ad[:, 128:160]), bv(ad[:, 160:192])
                s1v = scr1[:].rearrange("p (i j t) -> p i j t", i=32, j=16, t=2)
                s2v = scr2[:].rearrange("p (i j t) -> p i j t", i=32, j=16, t=2)
                sre = st[c][:, 0:DIM].rearrange("p (i j t) -> p i j t", i=32, j=16, t=2)
                sim = st[c][:, DIM : 2 * DIM].rearrange("p (i j t) -> p i j t", i=32, j=16, t=2)
                ttd(s1v, arv, brv, op=AOT.mult)
                ttd(s2v, aiv, biv, op=AOT.mult)
                ttd(sre, s1v, s2v, op=AOT.subtract)
                ttd(s1v, arv, biv, op=AOT.mult)
                ttd(s2v, aiv, brv, op=AOT.mult)
                ttd(sim, s1v, s2v, op=AOT.add)
            return X

        # ================= gates =================
        def emit_crx(X, c, ctrl, tgt, col, form):
            cc = X.ch[:, col : col + 1]
            ss = X.sh[:, col : col + 1]
            S = X.st[c][:]
            q, ht, half, tmaj, wx, psv = _crx_geom(S, ctrl, tgt)
            if form[0] == "PE":
                dC = pool.tile([P, 128], F16, tag="dC", name="dC")
                dSP = pool.tile([P, 256], F16, tag="dSP", name="dSP")
                tsd(dC[:], mask, cc)
                tsd(dSP[:], maskPM, ss)
                ps_ctr[0] ^= 1
                tag = "psA" if ps_ctr[0] else "psB"
                PT = pspool.tile([P, DIM], F32, tag=tag, name=tag)
                pt = PT[:]
                for tb in range(2):
                    mm(pt[:, 512 * tb : 512 * tb + 512], dC[:], ht(tb), True, False)
                for tb in range(2):
                    mm(pt[:, 512 * tb : 512 * tb + 256], dSP[:, 0:128], q(1, 1 - tb), False, True)
                for tb in range(2):
                    mm(pt[:, 512 * tb + 256 : 512 * tb + 512], dSP[:, 128:256], q(0, 1 - tb), False, True)
                ev = form[1]
                if ev == "A":
                    nc.scalar.copy(tmaj(), psv(pt))
                elif ev == "M":
                    nc.gpsimd.dma_start(tmaj(), psv(pt))
                else:
                    nc.vector.tensor_copy(tmaj(), psv(pt))
            else:
                Wt = pool.tile([P, DIM], F16, tag="crxW", name="crxW")
                Xt = pool.tile([P, DIM], F16, tag="crxX", name="crxX")
                wv = wx(Wt[:])
                xv = wx(Xt[:])
                ts_auto(wv, half(), cc)
                ts_auto(xv, half(plrev=True, trev=True), ss)
                hre = half()[:, 0]
                him = half()[:, 1]
                wre, wim = wv[:, 0], wv[:, 1]
                xre, xim = xv[:, 0], xv[:, 1]
                e0, e1 = form[1][0], form[1][1]
                (ttd if e0 == "D" else ttp)(hre, wre, xre, op=AOT.add)
                (ttd if e1 == "D" else ttp)(him, wim, xim, op=AOT.subtract)

        def emit_rot(X, c, w):
            """PE diag-matmul rot; psum layout (t, pl, o, i)."""
            u1 = X.U[1]
            j = 10 * c + w
            inner = 1 << (NQ - 1 - w)
            outer = HD // inner
            S = X.st[c][:]
            sv = S.rearrange("p (pl o t i) -> p pl o t i", pl=2, o=outer, t=2, i=inner)
            qv = lambda pl, tb: sv[:, pl, :, tb, :]
            dP = pool.tile([P, 128], F16, tag="dP", name="dP")
            dQ = pool.tile([P, 256], F16, tag="dQ", name="dQ")
            dR = pool.tile([P, 256], F16, tag="dR", name="dR")
            dS = pool.tile([P, 256], F16, tag="dS", name="dS")
            tsd(dP[:], mask, u1["p"][:, j : j + 1])
            tsd(dQ[:], maskPM, u1["q"][:, j : j + 1])    # [q | -q]
            tsd(dR[:], maskPM, u1["nr"][:, j : j + 1])   # [nr | r]
            tsd(dS[:], maskPM, u1["s"][:, j : j + 1])    # [s | -s]
            # all 16 MMs first (they read S), then the two half-evicts
            # (which overwrite S in place).
            PTs = []
            for tb in range(2):
                tag = "psA" if tb == 0 else "psB"
                PT = pspool.tile([P, DIM], F32, tag=tag, name=tag)
                PTs.append(PT)
                chunk = lambda pl, PT=PT: PT[:, pl * 512 : pl * 512 + 512]
                for pl in range(2):
                    mm(chunk(pl), dP[:], qv(pl, tb), True, False)
                # r-group: out[*,0] += r*S[*,1]; out[*,1] += nr*S[*,0]
                dRh = dR[:, 128:256] if tb == 0 else dR[:, 0:128]
                for pl in range(2):
                    mm(chunk(pl), dRh, qv(pl, 1 - tb), False, False)
                # s-group: out[im,t] += s*S[re,1-t]; out[re,t] += -s*S[im,1-t]
                mm(chunk(1), dS[:, 0:128], qv(0, 1 - tb), False, False)
                mm(chunk(0), dS[:, 128:256], qv(1, 1 - tb), False, False)
                # q-group (stop): +q on (re,1)/(im,0); -q on (re,0)/(im,1)
                mm(chunk(0), dQ[:, 0:128] if tb == 1 else dQ[:, 128:256],
                   qv(1, tb), False, True)
                mm(chunk(1), dQ[:, 0:128] if tb == 0 else dQ[:, 128:256],
                   qv(0, tb), False, True)
            for tb in range(2):
                dst = sv[:, :, :, tb, :]
                src = PTs[tb][:].rearrange(
                    "p (pl o i) -> p pl o i", pl=2, o=outer, i=inner
                )
                nc.scalar.copy(dst, src)

        # ================= tail =================
        def gen_tail(X):
            """Tail (combine + measure + output DMA) as a thunk list."""
            st = X.st
            scr1 = X.scr1
            acc = spool.tile([P, 2 * DIM], F16, tag="acc", name="acc")
            GG = pool.tile([P, NQ], F32, tag="GG", name="GG")
            cA = pool.tile([P, NQ], F32, tag="cA", name="cA")
            cB = pool.tile([P, NQ], F32, tag="cB", name="cB")
            hZ = pool.tile([P, NQ], F32, tag="hZ", name="hZ")
            scol = pool.tile([P, 8], F32, tag="scol", name="scol")
            msc32 = pool.tile([P, 2 * DIM], F32, tag="msc32", name="msc32")
            mscr = pool.tile([P, DIM], F16, tag="mscr", name="mscr")
            cfc = lambda k: cf_t[:, k : k + 1]

            def combine():
                w3 = pool.tile([P, 2 * DIM], F16, tag="w3", name="w3")
                w4 = pool.tile([P, 2 * DIM], F16, tag="w4", name="w4")
                if real_cf:
                    tsd(acc[:], st[0][:], cfc(0))
                    tsd(w3[:], st[1][:], cfc(3))
                    tsd(w4[:], st[2][:], cfc(6))
                    ttp(acc[:], acc[:], w3[:], op=AOT.add)
                    ttd(acc[:], acc[:], w4[:], op=AOT.add)
                else:
                    for pl in range(2):
                        out_sl = acc[:, pl * DIM : (pl + 1) * DIM]
                        for k in range(3):
                            s_re = cfc(3 * k) if pl == 0 else cfc(3 * k + 1)
                            s_im = cfc(3 * k + 2) if pl == 0 else cfc(3 * k)
                            dst = out_sl if k == 0 else scr1[:]
                            ax(dst, st[k][:, 0:DIM], s_re,
                               st[k][:, DIM : 2 * DIM], s_im)
                            if k > 0:
                                ttd(out_sl, out_sl, scr1[:], op=AOT.add)
                nc.scalar.activation(msc32[:], acc[:], ACTF.Square, accum_out=scol[:, 0:1])

            def _wire_views(plane, w):
                inner = 1 << (NQ - 1 - w)
                outer = HD // inner
                v = plane.rearrange("p (o t i) -> p o t i", o=outer, t=2, i=inner)
                return v[:, :, 0, :], v[:, :, 1, :]

            def meas_wire(w):
                inner = 1 << (NQ - 1 - w)
                outer = HD // inner
                accr, acci = acc[:, 0:DIM], acc[:, DIM : 2 * DIM]
                fv = acc[:].rearrange(
                    "p (m t i) -> p m t i", m=2 * outer, t=2, i=inner
                )
                p0b = fv[:, :, 0, :]
                p1b = fv[:, :, 1, :]
                ms2 = mscr[:].rearrange("p (m i) -> p m i", m=2 * outer, i=inner)
                ttd(ms2, p0b, p1b, op=AOT.add)
                nc.scalar.activation(
                    msc32[:, 0:DIM], mscr[:], ACTF.Square,
                    accum_out=GG[:, w : w + 1],
                )
                nc.scalar.activation(
                    msc32[:, DIM : 2 * DIM].rearrange(
                        "p (m i) -> p m i", m=2 * outer, i=inner
                    ),
                    p1b, ACTF.Square, accum_out=hZ[:, w : w + 1],
                )
                p0r, p1r = _wire_views(accr, w)
                p0i, p1i = _wire_views(acci, w)
                ms1 = mscr[:, 0:HD].rearrange("p (o i) -> p o i", o=outer, i=inner)
                nc.vector.scalar_tensor_tensor(
                    ms1, p0r, 0.0, p1i, op0=AOT.bypass, op1=AOT.mult,
                    accum_out=cA[:, w : w + 1],
                )
                nc.vector.scalar_tensor_tensor(
                    ms1, p0i, 0.0, p1r, op0=AOT.bypass, op1=AOT.mult,
                    accum_out=cB[:, w : w + 1],
                )

            def finalize():
                nc.vector.tensor_scalar(
                    scol[:, 1:2], scol[:, 0:1], 1e-9, None, op0=AOT.add
                )
                nc.vector.reciprocal(scol[:, 2:3], scol[:, 1:2])
                nc.vector.tensor_scalar(scol[:, 3:4], scol[:, 2:3], 2.0, None, op0=AOT.mult)
                nc.vector.tensor_scalar(scol[:, 4:5], scol[:, 2:3], -2.0, None, op0=AOT.mult)
                ttd(scol[:, 5:6], scol[:, 0:1], scol[:, 2:3], op=AOT.mult)
                nc.vector.tensor_scalar(scol[:, 6:7], scol[:, 5:6], -1.0, None, op0=AOT.mult)
                out30 = pool.tile([P, 30], F32, tag="out30", name="out30")
                wv_ = pool.tile([P, 10], F32, tag="wv", name="wv")
                nszb = scol[:, 6:7].broadcast_to([P, 1, NQ])
                nc.vector.scalar_tensor_tensor(
                    out30[:, 0:10].unsqueeze(1), GG[:].unsqueeze(1), scol[:, 2:3], nszb,
                    op0=AOT.mult, op1=AOT.add,
                )
                ttd(wv_[:], cA[:], cB[:], op=AOT.subtract)
                tsd(out30[:, 10:20], wv_[:], scol[:, 3:4])
                szb = scol[:, 5:6].broadcast_to([P, 1, NQ])
                nc.vector.scalar_tensor_tensor(
                    out30[:, 20:30].unsqueeze(1), hZ[:].unsqueeze(1), scol[:, 4:5], szb,
                    op0=AOT.mult, op1=AOT.add,
                )
                nc.sync.dma_start(out_d[X.r0:X.r1, :], out30[:])

            thunks = [combine]
            for w in range(NQ):
                thunks.append(lambda w=w: meas_wire(w))
            thunks.append(finalize)
            return thunks

        # ================= thunk generators =================
        def gen_rings(X, L):
            rings = [_ring_gates(c, L) for c in range(3)]
            thunks = []
            for k in range(2 * NQ):
                for c in range(3):
                    ctrl, tgt, col = rings[c][k]
                    def th(X=X, c=c, ctrl=ctrl, tgt=tgt, col=col):
                        kk = crx_ctr[0]
                        crx_ctr[0] += 1
                        emit_crx(X, c, ctrl, tgt, col, CRX_PATTERN[kk % len(CRX_PATTERN)])
                    thunks.append(th)
            return thunks

        def gen_rot(X):
            thunks = []
            for w in range(NQ):
                for c in range(3):
                    thunks.append(lambda X=X, c=c, w=w: emit_rot(X, c, w))
            return thunks

        def weave(*lists):
            """Proportional round-robin emission of thunk lists."""
            lists = [l for l in lists if l]
            if not lists:
                return
            total = max(len(l) for l in lists)
            idx = [0] * len(lists)
            for step in range(total):
                for li, l in enumerate(lists):
                    want = (step + 1) * len(l) // total
                    while idx[li] < want:
                        l[idx[li]]()
                        idx[li] += 1

        # ================= pipelined schedule =================
        # phases per tile: P prologue, A rings0, B rot, C rings1, D tail.
        # Emission order (each phase exactly once, ~3 tiles in flight):
        #   P0 P1 P2 A0 [B0|A1] then per k:
        #     P(k+3), [C(k) | B(k+1) | A(k+2) | D(k-1)]
        # and D(n-1) at the end.
        n = n_tiles
        ctxs = {}
        ctxs[0] = emit_prologue(0)
        if n > 1:
            ctxs[1] = emit_prologue(1)
        weave(gen_rings(ctxs[0], 0))
        if n > 2:
            ctxs[2] = emit_prologue(2)
        weave(gen_rot(ctxs[0]), gen_rings(ctxs[1], 0) if n > 1 else [])
        for k in range(n):
            if k + 3 < n:
                ctxs[k + 3] = emit_prologue(k + 3)
            weave(
                gen_rings(ctxs[k], 1),
                gen_rot(ctxs[k + 1]) if k + 1 < n else [],
                gen_rings(ctxs[k + 2], 0) if k + 2 < n else [],
                gen_tail(ctxs[k - 1]) if k >= 1 else [],
            )
        weave(gen_tail(ctxs[n - 1]))


def build_nc(n_tiles=NT, b_core=None, real_cf=True):
    if b_core is None:
        b_core = n_tiles * P
    nc = bacc.Bacc("TRN2", target_bir_lowering=False)
    ins = {
        "ps": nc.dram_tensor("ps", [b_core, NCOL], F32, kind="ExternalInput")[:],
        "dth": nc.dram_tensor("dth", [b_core, 1], F32, kind="ExternalInput")[:],
        "cf": nc.dram_tensor("cf", [P, 9], F32, kind="ExternalInput")[:],
        "masks": nc.dram_tensor("masks", [P, 384], F16, kind="ExternalInput")[:],
    }
    outs = {"out": nc.dram_tensor("out", [b_core, 30], F32, kind="ExternalOutput")[:]}
    with tile.TileContext(nc) as tc:
        emit_core_kernel(nc, tc, ins, outs, n_tiles=n_tiles, real_cf=real_cf)
    nc.compile()
    return nc


_NC_CACHE = {}


def _get_nc(n_tiles=NT, real_cf=True):
    key = (n_tiles, real_cf)
    if key not in _NC_CACHE:
        _NC_CACHE[key] = build_nc(n_tiles, real_cf=real_cf)
    return _NC_CACHE[key]


def make_host_inputs(input_angles, forward_params, backward_params, diagonal_params,
                     dt_scale, alpha_real, alpha_imag, beta_real, beta_imag,
                     gamma_real, gamma_imag):
    """Host-side scalar prep + param column rearrangement."""
    al = complex(float(alpha_real), float(alpha_imag))
    be = complex(float(beta_real), float(beta_imag))
    ga = complex(float(gamma_real), float(gamma_imag))
    n = np.sqrt(abs(al) ** 2 + abs(be) ** 2 + abs(ga) ** 2 + 1e-9)
    cs = [al / n, be / n, ga / n]
    row = []
    for ck in cs:
        row += [ck.real, ck.imag, -ck.imag]
    cf = np.tile(np.asarray(row, np.float32), (P, 1))
    dth = (0.25 * np.asarray(dt_scale, np.float32)).reshape(-1, 1)

    B = np.asarray(input_angles).shape[0]
    ps = np.empty((B, NCOL), np.float32)
    pars = [np.asarray(forward_params, np.float32),
            np.asarray(backward_params, np.float32),
            np.asarray(diagonal_params, np.float32)]
    for c in range(3):
        par = pars[c]
        for L in range(2):
            base = 50 * L
            for w in range(NQ):
                g = w if c != BWD else NQ - 1 - w
                ps[:, rot_col(c, L, w, 0) : rot_col(c, L, w, 0) + 3] = (
                    par[:, base + 3 * g : base + 3 * g + 3]
                )
            ps[:, crx_col(c, L, 0) : crx_col(c, L, 0) + 20] = (
                par[:, base + 30 : base + 50]
            )
    ps[:, ANG0:ANG0 + NQ] = np.asarray(input_angles, np.float32)

    eye = np.eye(P, dtype=np.float16)
    masks = np.concatenate([eye, eye, -eye], axis=1)
    return cf, dth, ps, masks


def kernel(**inputs):
    from concourse.bass_utils import run_bass_kernel_spmd

    cf, dth, ps, masks = make_host_inputs(**inputs)
    real_cf = (
        abs(float(inputs["alpha_imag"])) < 1e-30
        and abs(float(inputs["beta_imag"])) < 1e-30
        and abs(float(inputs["gamma_imag"])) < 1e-30
    )
    nc = _get_nc(NT, real_cf)
    in_maps = []
    for c in range(N_CORES):
        r0, r1 = c * B_CORE, (c + 1) * B_CORE
        in_maps.append({
            "ps": np.ascontiguousarray(ps[r0:r1]),
            "dth": np.ascontiguousarray(dth[r0:r1]),
            "cf": cf,
            "masks": masks,
        })
    res = run_bass_kernel_spmd(nc, in_maps, core_ids=list(range(N_CORES)))
    out = np.concatenate([res.results[c]["out"] for c in range(N_CORES)], axis=0)
    return out.astype(np.float32)


# revision 41
# speedup vs baseline: 1.0057x; 1.0057x over previous
"""Trainium2 Bass kernel for ClassicalReconstructionHydraSSMCore.

Quantum statevector simulation: batch 8192, 10 qubits, three circuits
(forward/backward/diagonal), combine + normalize + Pauli X/Y/Z measure.

Sharding: pure data parallel over batch across 8 cores (1024 each).
Per-core layout: batch on partitions (8 tiles of 128), state on free dim
as fp16 [re(1024) | im(1024)].

v2 design (cost-model driven):
 - rot gates (folded RZ*RY*RX per wire) run on the Tensor engine as
   diagonal-weight matmuls: per-batch scalars become diag(u) 128x128
   weights, terms accumulate in PSUM fp32, then one evict op converts
   back to fp16 SBUF.
 - CRX gates are striped between a 4-op DVE form (two 4x-mode
   tensor_scalar partials + two 2x tensor_tensor combines, partly on
   Pool) and the PE diag-matmul form.
 - Tiles are software-pipelined: rot(t) [PE-heavy] is interleaved with
   rings0(t+1) [DVE/Pool-heavy], rings1(t) with rot(t+1), tail(t) with
   rings0(t+2), so no engine starves during phase transitions.
"""

import numpy as np

import concourse.bass as bass
import concourse.tile as tile
from concourse import bacc, mybir

F32 = mybir.dt.float32
F16 = mybir.dt.float16
AOT = mybir.AluOpType
ACTF = mybir.ActivationFunctionType


def _register_axpby():
    """Runtime-register a custom DVE op: out = in0*s0 + in1*s1."""
    import concourse.dve_ops as dve_ops
    from concourse.dve_spec import Spec, Src0, Src1, C0, C1, lower
    from concourse.dve_spec import _has_src1 as has_src1
    from concourse.dve_uop import DveOpSpec

    name = "AXPBY9_ANT"
    for op in dve_ops.OPS:
        if op.name == name:
            return op
    spec = Spec(
        body=Src0 * C0 + Src1 * C1,
        reference=lambda in0, in1, s0, s1, imm2: in0 * s0 + in1 * s1,
    )
    row = dve_ops._CUSTOM_DVE_ROW_BASE + len(dve_ops.OPS)
    assert row < 0x20
    dve_ops._SUB_OPCODE_FOR_NAME[name] = row
    shas = {}
    for ver in ("v3", "v4"):
        s = DveOpSpec(
            name=name, opcode=row, uops=lower(spec, ver=ver), rd1_en=has_src1(spec)
        )
        shas[ver] = s.sha(ver)
    op = dve_ops.DveOp(name, spec, subdim=False, uops_sha=shas)
    dve_ops.OPS.append(op)
    dve_ops.CUSTOM_DVE_SPECS[name] = spec
    return op


AXPBY = _register_axpby()

NQ = 10
DIM = 1 << NQ          # 1024
HD = DIM // 2          # 512
P = 128
N_CORES = 8
B_CORE = 1024
NT = B_CORE // P       # 8 tiles per core
PI_2 = float(np.pi / 2)

FWD, BWD, DIAG = 0, 1, 2

# param column layout on device (310 cols):
#  rot block [0,180): col(c,L,w,k) = 60c+30L+3w+k, wire-indexed for all
#    circuits (host rearrange absorbs BWD's reversed wire order).
#    cols [0,120) (fwd+bwd) are dt-scaled, [120,180) (diag) are not.
#  crx block [180,300): col(c,L,j) = 180+40c+20L+j, j = time order.
#  angles [300,310).
NCOL = 310
CRX0 = 180
ANG0 = 300


def rot_col(c, L, w, k):
    return 60 * c + 30 * L + 3 * w + k


def crx_col(c, L, j):
    return CRX0 + 40 * c + 20 * L + j


def _ring_gates(c, L):
    """Time-ordered entangler list [(ctrl, tgt, col)] for circuit c, layer L."""
    out = []
    if c in (FWD, DIAG):
        for k in range(NQ):       # ring1: CRX(i, i+1), i ascending
            out.append((k, (k + 1) % NQ, crx_col(c, L, k)))
        for k in range(NQ):       # ring2: CRX(i, i-1), i descending
            i = NQ - 1 - k
            out.append((i, (i - 1) % NQ, crx_col(c, L, NQ + k)))
    else:  # BWD
        for k in range(NQ):       # ring1: CRX(i, i-1), i descending
            i = NQ - 1 - k
            out.append((i, (i - 1) % NQ, crx_col(c, L, k)))
        for k in range(NQ):       # ring2: CRX(i, i+1), i ascending
            out.append((k, (k + 1) % NQ, crx_col(c, L, NQ + k)))
    return out


def _crx_geom(S, ctrl, tgt):
    """Views for a CRX(ctrl,tgt) gate on state tile AP S (P, 2048)."""
    hi, lo = (ctrl, tgt) if ctrl < tgt else (tgt, ctrl)
    if lo - hi == 1:
        a = 1 << hi
        z = 1 << (8 - hi)
        v = S.rearrange("p (pl a x y z) -> p pl a x y z", pl=2, a=a, x=2, y=2, z=z)
        if ctrl == hi:
            q = lambda pl, t: v[:, pl, :, 1, t, :]
            ht = lambda t: v[:, :, :, 1, t, :]
            def half(plrev=False, trev=False):
                h = v[:, :, :, 1, :, :]      # (pl, a, t, z)
                if plrev:
                    h = h[:, ::-1]
                if trev:
                    h = h[:, :, :, ::-1, :]
                return h
        else:
            q = lambda pl, t: v[:, pl, :, t, 1, :]
            ht = lambda t: v[:, :, :, t, 1, :]
            def half(plrev=False, trev=False):
                h = v[:, :, :, :, 1, :]      # (pl, a, t, z)
                if plrev:
                    h = h[:, ::-1]
                if trev:
                    h = h[:, :, :, ::-1, :]
                return h
        tmaj = lambda: half().transpose([0, 3, 1, 2, 4])
        wx = lambda W: W.rearrange("p (pl a t z) -> p pl a t z", pl=2, a=a, t=2, z=z)
        psv = lambda pt: pt.rearrange("p (t pl a z) -> p t pl a z", t=2, pl=2, a=a, z=z)
    else:
        b = DIM // 4
        v = S.rearrange("p (pl x b y) -> p pl x b y", pl=2, x=2, b=b, y=2)
        if ctrl == 0:
            # ctrl-dim = x, tgt-dim = y; half dims (pl, b, t)
            q = lambda pl, t: v[:, pl, 1, :, t]
            ht = lambda t: v[:, :, 1, :, t]
            def half(plrev=False, trev=False):
                h = v[:, :, 1, :, :]         # (pl, b, t)
                if plrev:
                    h = h[:, ::-1]
                if trev:
                    h = h[:, :, :, ::-1]
                return h
            tmaj = lambda: half().transpose([0, 3, 1, 2])
            wx = lambda W: W.rearrange("p (pl b t) -> p pl b t", pl=2, b=b, t=2)
        else:
            # ctrl == NQ-1 (dim y), tgt-dim = x; half dims (pl, t, b)
            q = lambda pl, t: v[:, pl, t, :, 1]
            ht = lambda t: v[:, :, t, :, 1]
            def half(plrev=False, trev=False):
                h = v[:, :, :, :, 1]         # (pl, t, b)
                if plrev:
                    h = h[:, ::-1]
                if trev:
                    h = h[:, :, ::-1, :]
                return h
            tmaj = lambda: half().transpose([0, 2, 1, 3])
            wx = lambda W: W.rearrange("p (pl t b) -> p pl t b", pl=2, t=2, b=b)
        psv = lambda pt: pt.rearrange("p (t pl b) -> p t pl b", t=2, pl=2, b=b)
    return q, ht, half, tmaj, wx, psv


class _Ctx:
    pass


def emit_core_kernel(nc, tc, ins, outs, n_tiles=NT, real_cf=True):
    ps_d = ins["ps"]
    dth_d = ins["dth"]
    cf_d = ins["cf"]
    msk_d = ins["masks"]
    out_d = outs["out"]

    tsd = nc.vector.tensor_scalar_mul
    ttd = nc.vector.tensor_tensor
    ttp = nc.gpsimd.tensor_tensor
    ax = lambda out, x, sx, y, sy: nc.vector._custom_dve(
        AXPBY, out=out, in0=x, in1=y, s0=sx, s1=sy
    )

    # gate->engine striping (tuned via TimelineSim):
    #  ("PE", e): diag-matmul form; e = evict engine 'A'/'D'/'P'
    #  ("DV", e0e1): DVE partials; combines on e0 (re) / e1 (im), 'D'/'P'
    CRX_PATTERN = [
        ("PE", "A"), ("DV", "DP"), ("PE", "A"), ("DV", "DP"),
        ("PE", "A"), ("DV", "DP"), ("DV", "PP"), ("DV", "DP"),
    ]
    crx_ctr = [0]
    ps_ctr = [0]

    with (
        tc.tile_pool(name="const", bufs=1) as cpool,
        tc.tile_pool(name="work", bufs=3) as pool,
        tc.tile_pool(name="state", bufs=4) as spool,
        tc.tile_pool(name="psum", bufs=2, space="PSUM") as pspool,
    ):
        cf_t = cpool.tile([P, 16], F32)
        nc.sync.dma_start(cf_t[:, 0 : cf_d.shape[1]], cf_d[:])
        msk = cpool.tile([P, 384], F16)
        nc.sync.dma_start(msk[:], msk_d[:])
        mask = msk[:, 0:128]       # identity
        maskPM = msk[:, 128:384]   # [I | -I]
        pi2 = cpool.tile([P, 1], F32)
        nc.gpsimd.memset(pi2[:], PI_2)
        pi2c = pi2[:, 0:1]

        def _nfree(ap):
            return len(ap.opt().ap) - 1

        def ts_auto(out_v, in_v, sc):
            """TS, split along the plane dim if >3 free dims after opt."""
            if _nfree(out_v) <= 3 and _nfree(in_v) <= 3:
                tsd(out_v, in_v, sc)
            else:
                for pl in range(2):
                    tsd(out_v[:, pl], in_v[:, pl], sc)

        def mm(out_ap, w_ap, mov_ap, start, stop):
            nc.tensor.matmul(out_ap, w_ap, mov_ap, start=start, stop=stop)

        # ================= prologue =================
        def emit_prologue(t):
            X = _Ctx()
            X.t = t
            r0, r1 = t * P, (t + 1) * P
            X.r0, X.r1 = r0, r1
            ps = pool.tile([P, NCOL], F32, tag="ps")
            nc.sync.dma_start(ps[:], ps_d[r0:r1, :])
            dth = pool.tile([P, 1], F32, tag="dth")
            nc.sync.dma_start(dth[:], dth_d[r0:r1, :])

            sh = pool.tile([P, NCOL], F32, tag="sh")
            ch = pool.tile([P, NCOL], F32, tag="ch")
            trA = pool.tile([P, NCOL], F32, tag="trA")
            trB = pool.tile([P, NCOL], F32, tag="trB")
            nc.scalar.activation(sh[:, 0:120], ps[:, 0:120], ACTF.Sin, scale=dth[:, 0:1])
            nc.scalar.activation(sh[:, 120:NCOL], ps[:, 120:NCOL], ACTF.Sin, scale=0.25)
            nc.scalar.activation(
                ch[:, 0:120], ps[:, 0:120], ACTF.Sin, scale=dth[:, 0:1], bias=pi2c
            )
            nc.scalar.activation(
                ch[:, 120:NCOL], ps[:, 120:NCOL], ACTF.Sin, scale=0.25, bias=pi2c
            )
            ttp(trA[:], sh[:], ch[:], op=AOT.mult)
            ttp(trB[:], sh[:], sh[:], op=AOT.mult)
            nc.gpsimd.tensor_scalar_mul(sh[:], trA[:], 2.0)
            nc.gpsimd.tensor_scalar(ch[:], trB[:], -2.0, 1.0, op0=AOT.mult, op1=AOT.add)
            X.sh, X.ch = sh, ch
            X.pro2 = []

            # product-state tiles allocated up front (ring rotation order)
            st = [spool.tile([P, 2 * DIM], F16, tag=f"st{c}", name=f"st{c}_{t}") for c in range(3)]
            X.st = st
            scr1 = pool.tile([P, DIM], F16, tag="scr1")
            scr2 = pool.tile([P, DIM], F16, tag="scr2")
            X.scr1 = scr1

            def pro_ucoef_v():
                self_build_ucoef_v(X)

            X.pro2.append(pro_ucoef_v)

            def pro_circuits():
                ads = [self_build_circuit(X, c, scr1, scr2) for c in range(3)]
                for c in range(3):
                    self_build_outer(X, c, ads[c], scr1, scr2)

            X.pro2.append(pro_circuits)
            return X

        def self_build_ucoef_v(X):
            sh, ch = X.sh, X.ch
            t = X.t
            # u-coefficients per layer
            rotc = ch[:, 0:180].rearrange("p (c L w k) -> p c L w k", c=3, L=2, w=10, k=3)
            rots = sh[:, 0:180].rearrange("p (c L w k) -> p c L w k", c=3, L=2, w=10, k=3)
            m1 = pool.tile([P, 30], F32, tag="m1")
            m2 = pool.tile([P, 30], F32, tag="m2")
            m3 = pool.tile([P, 30], F32, tag="m3")
            m4 = pool.tile([P, 30], F32, tag="m4")
            w1 = pool.tile([P, 30], F32, tag="w1")
            w2 = pool.tile([P, 30], F32, tag="w2")
            V = lambda tl: tl[:].rearrange("p (c g) -> p c g", c=3, g=10)
            U = []
            for L in range(2):
                ca = rotc[:, :, L, :, 0]
                cb = rotc[:, :, L, :, 1]
                cg = rotc[:, :, L, :, 2]
                sa = rots[:, :, L, :, 0]
                sb = rots[:, :, L, :, 1]
                sg = rots[:, :, L, :, 2]
                u = {
                    k: pool.tile([P, 30], F32, tag=f"u{k}{L}", name=f"u{k}{L}_{t}")
                    for k in ("p", "q", "nr", "s")
                }
                ttp(V(m1), cb, ca, op=AOT.mult)
                ttp(V(m2), sb, sa, op=AOT.mult)
                ttp(V(m3), sb, ca, op=AOT.mult)
                ttp(V(m4), cb, sa, op=AOT.mult)
                ttp(V(w1), cg, V(m1), op=AOT.mult)
                ttp(V(w2), sg, V(m2), op=AOT.mult)
                ttp(V(u["p"]), V(w1), V(w2), op=AOT.add)
                ttp(V(w1), cg, V(m2), op=AOT.mult)
                ttp(V(w2), sg, V(m1), op=AOT.mult)
                ttp(V(u["q"]), V(w1), V(w2), op=AOT.subtract)
                ttp(V(w1), cg, V(m3), op=AOT.mult)
                ttp(V(w2), sg, V(m4), op=AOT.mult)
                ttp(V(u["nr"]), V(w1), V(w2), op=AOT.add)
                ttp(V(w1), sg, V(m3), op=AOT.mult)
                ttp(V(w2), cg, V(m4), op=AOT.mult)
                ttp(V(u["s"]), V(w1), V(w2), op=AOT.subtract)
                U.append(u)
            X.U = U

            # v vectors: layer-0 rotations folded into init
            u0 = U[0]
            angc = ch[:, ANG0:ANG0 + 10]
            angs = sh[:, ANG0:ANG0 + 10]
            a3c = pool.tile([P, 30], F32, tag="a3c")
            a3s = pool.tile([P, 30], F32, tag="a3s")
            for c in range(3):
                nc.scalar.copy(a3c[:, 10 * c : 10 * c + 10], angc)
                nc.scalar.copy(a3s[:, 10 * c : 10 * c + 10], angs)
            v0r = pool.tile([P, 30], F32, tag="v0r")
            v0i = pool.tile([P, 30], F32, tag="v0i")
            v1r = pool.tile([P, 30], F32, tag="v1r")
            v1i = pool.tile([P, 30], F32, tag="v1i")
            nv0i = pool.tile([P, 30], F32, tag="nv0i")
            nv1i = pool.tile([P, 30], F32, tag="nv1i")
            ttp(w1[:], u0["p"][:], a3c[:], op=AOT.mult)
            ttp(w2[:], u0["nr"][:], a3s[:], op=AOT.mult)
            ttp(v0r[:], w1[:], w2[:], op=AOT.subtract)
            ttp(w1[:], u0["q"][:], a3c[:], op=AOT.mult)
            ttp(w2[:], u0["s"][:], a3s[:], op=AOT.mult)
            ttp(v0i[:], w1[:], w2[:], op=AOT.add)
            ttp(w1[:], u0["nr"][:], a3c[:], op=AOT.mult)
            ttp(w2[:], u0["p"][:], a3s[:], op=AOT.mult)
            ttp(v1r[:], w1[:], w2[:], op=AOT.add)
            ttp(w1[:], u0["s"][:], a3c[:], op=AOT.mult)
            ttp(w2[:], u0["q"][:], a3s[:], op=AOT.mult)
            ttp(v1i[:], w1[:], w2[:], op=AOT.subtract)
            nc.gpsimd.tensor_scalar_mul(nv0i[:], v0i[:], -1.0)
            nc.gpsimd.tensor_scalar_mul(nv1i[:], v1i[:], -1.0)
            X.v = (v0r, v0i, v1r, v1i, nv0i, nv1i)

        def self_build_circuit(X, c, scr1, scr2):
            """Expand product state + outer product for circuit c."""
            v0r, v0i, v1r, v1i, nv0i, nv1i = X.v
            st = X.st
            ab = [pool.tile([P, 32], F32, tag=f"ab{c}_{k}", name=f"ab{c}_{k}") for k in range(8)]
            adup = pool.tile([P, 192], F16, tag=f"adup{c}", name=f"adup{c}")

            def expand(bufs, wires):
                br, bi, br2, bi2 = bufs
                j0 = 10 * c + wires[0]
                for dst, src in (
                    (br[:, 0:1], v0r), (br[:, 1:2], v1r),
                    (bi[:, 0:1], v0i), (bi[:, 1:2], v1i),
                ):
                    tsd(dst, src[:, j0 : j0 + 1], 1.0)
                width = 2
                cur_r, cur_i, oth_r, oth_i = br, bi, br2, bi2
                for w in wires[1:]:
                    j = 10 * c + w
                    c0r, c0i = v0r[:, j : j + 1], v0i[:, j : j + 1]
                    c1r, c1i = v1r[:, j : j + 1], v1i[:, j : j + 1]
                    n0i, n1i = nv0i[:, j : j + 1], nv1i[:, j : j + 1]
                    old_r, old_i = cur_r[:, 0:width], cur_i[:, 0:width]
                    nw = 2 * width
                    nr_v = oth_r[:, 0:nw].rearrange("p (w t) -> p w t", w=width, t=2)
                    ni_v = oth_i[:, 0:nw].rearrange("p (w t) -> p w t", w=width, t=2)
                    ax(nr_v[:, :, 0], old_r, c0r, old_i, n0i)
                    ax(ni_v[:, :, 0], old_r, c0i, old_i, c0r)
                    ax(nr_v[:, :, 1], old_r, c1r, old_i, n1i)
                    ax(ni_v[:, :, 1], old_r, c1i, old_i, c1r)
                    cur_r, oth_r = oth_r, cur_r
                    cur_i, oth_i = oth_i, cur_i
                    width = nw
                return cur_r, cur_i

            ar, ai = expand(ab[0:4], list(range(5)))
            br_, bi_ = expand(ab[4:8], list(range(5, NQ)))
            ad = adup
            nc.scalar.copy(
                ad[:, 0:64].rearrange("p (i t) -> p i t", i=32, t=2),
                ar[:, 0:32].rearrange("p (i t) -> p i t", i=32, t=1).broadcast_to([P, 32, 2]),
            )
            nc.scalar.copy(
                ad[:, 64:128].rearrange("p (i t) -> p i t", i=32, t=2),
                ai[:, 0:32].rearrange("p (i t) -> p i t", i=32, t=1).broadcast_to([P, 32, 2]),
            )
            nc.scalar.copy(ad[:, 128:160], br_[:, 0:32])
            nc.scalar.copy(ad[:, 160:192], bi_[:, 0:32])
            return adup

        def self_build_outer(X, c, adup, scr1, scr2):
            st = X.st
            ad = adup
            jv = lambda sl: sl.rearrange("p (i o t) -> p i o t", i=32, o=1, t=2).broadcast_to([P, 32, 16, 2])
            bv = lambda sl: sl.rearrange("p (o j t) -> p o j t", o=1, j=16, t=2).broadcast_to([P, 32, 16, 2])
            arv, aiv = jv(ad[:, 0:64]), jv(ad[:, 64:128])
            brv, biv = bv(ad[:, 128:160]), bv(ad[:, 160:192])
            s1v = scr1[:].rearrange("p (i j t) -> p i j t", i=32, j=16, t=2)
            s2v = scr2[:].rearrange("p (i j t) -> p i j t", i=32, j=16, t=2)
            sre = st[c][:, 0:DIM].rearrange("p (i j t) -> p i j t", i=32, j=16, t=2)
            sim = st[c][:, DIM : 2 * DIM].rearrange("p (i j t) -> p i j t", i=32, j=16, t=2)
            ttd(s1v, arv, brv, op=AOT.mult)
            ttd(s2v, aiv, biv, op=AOT.mult)
            ttd(sre, s1v, s2v, op=AOT.subtract)
            ttd(s1v, arv, biv, op=AOT.mult)
            ttd(s2v, aiv, brv, op=AOT.mult)
            ttd(sim, s1v, s2v, op=AOT.add)

        # ================= gates =================
        def emit_crx(X, c, ctrl, tgt, col, form):
            cc = X.ch[:, col : col + 1]
            ss = X.sh[:, col : col + 1]
            S = X.st[c][:]
            q, ht, half, tmaj, wx, psv = _crx_geom(S, ctrl, tgt)
            if form[0] == "PE":
                dC = pool.tile([P, 128], F16, tag="dC", name="dC")
                dSP = pool.tile([P, 256], F16, tag="dSP", name="dSP")
                tsd(dC[:], mask, cc)
                tsd(dSP[:], maskPM, ss)
                ps_ctr[0] ^= 1
                tag = "psA" if ps_ctr[0] else "psB"
                PT = pspool.tile([P, DIM], F32, tag=tag, name=tag)
                pt = PT[:]
                for tb in range(2):
                    mm(pt[:, 512 * tb : 512 * tb + 512], dC[:], ht(tb), True, False)
                for tb in range(2):
                    mm(pt[:, 512 * tb : 512 * tb + 256], dSP[:, 0:128], q(1, 1 - tb), False, True)
                for tb in range(2):
                    mm(pt[:, 512 * tb + 256 : 512 * tb + 512], dSP[:, 128:256], q(0, 1 - tb), False, True)
                ev = form[1]
                if ev == "A":
                    nc.scalar.copy(tmaj(), psv(pt))
                elif ev == "M":
                    nc.gpsimd.dma_start(tmaj(), psv(pt))
                else:
                    nc.vector.tensor_copy(tmaj(), psv(pt))
            else:
                Wt = pool.tile([P, DIM], F16, tag="crxW", name="crxW")
                Xt = pool.tile([P, DIM], F16, tag="crxX", name="crxX")
                wv = wx(Wt[:])
                xv = wx(Xt[:])
                ts_auto(wv, half(), cc)
                ts_auto(xv, half(plrev=True, trev=True), ss)
                hre = half()[:, 0]
                him = half()[:, 1]
                wre, wim = wv[:, 0], wv[:, 1]
                xre, xim = xv[:, 0], xv[:, 1]
                e0, e1 = form[1][0], form[1][1]
                (ttd if e0 == "D" else ttp)(hre, wre, xre, op=AOT.add)
                (ttd if e1 == "D" else ttp)(him, wim, xim, op=AOT.subtract)

        def emit_rot(X, c, w):
            """PE diag-matmul rot; psum layout (t, pl, o, i)."""
            u1 = X.U[1]
            j = 10 * c + w
            inner = 1 << (NQ - 1 - w)
            outer = HD // inner
            S = X.st[c][:]
            sv = S.rearrange("p (pl o t i) -> p pl o t i", pl=2, o=outer, t=2, i=inner)
            qv = lambda pl, tb: sv[:, pl, :, tb, :]
            dP = pool.tile([P, 128], F16, tag="dP", name="dP")
            dQ = pool.tile([P, 256], F16, tag="dQ", name="dQ")
            dR = pool.tile([P, 256], F16, tag="dR", name="dR")
            dS = pool.tile([P, 256], F16, tag="dS", name="dS")
            tsd(dP[:], mask, u1["p"][:, j : j + 1])
            tsd(dQ[:], maskPM, u1["q"][:, j : j + 1])    # [q | -q]
            tsd(dR[:], maskPM, u1["nr"][:, j : j + 1])   # [nr | r]
            tsd(dS[:], maskPM, u1["s"][:, j : j + 1])    # [s | -s]
            # all 16 MMs first (they read S), then the two half-evicts
            # (which overwrite S in place).
            PTs = []
            for tb in range(2):
                tag = "psA" if tb == 0 else "psB"
                PT = pspool.tile([P, DIM], F32, tag=tag, name=tag)
                PTs.append(PT)
                chunk = lambda pl, PT=PT: PT[:, pl * 512 : pl * 512 + 512]
                for pl in range(2):
                    mm(chunk(pl), dP[:], qv(pl, tb), True, False)
                # r-group: out[*,0] += r*S[*,1]; out[*,1] += nr*S[*,0]
                dRh = dR[:, 128:256] if tb == 0 else dR[:, 0:128]
                for pl in range(2):
                    mm(chunk(pl), dRh, qv(pl, 1 - tb), False, False)
                # s-group: out[im,t] += s*S[re,1-t]; out[re,t] += -s*S[im,1-t]
                mm(chunk(1), dS[:, 0:128], qv(0, 1 - tb), False, False)
                mm(chunk(0), dS[:, 128:256], qv(1, 1 - tb), False, False)
                # q-group (stop): +q on (re,1)/(im,0); -q on (re,0)/(im,1)
                mm(chunk(0), dQ[:, 0:128] if tb == 1 else dQ[:, 128:256],
                   qv(1, tb), False, True)
                mm(chunk(1), dQ[:, 0:128] if tb == 0 else dQ[:, 128:256],
                   qv(0, tb), False, True)
            for tb in range(2):
                dst = sv[:, :, :, tb, :]
                src = PTs[tb][:].rearrange(
                    "p (pl o i) -> p pl o i", pl=2, o=outer, i=inner
                )
                nc.scalar.copy(dst, src)

        # ================= tail =================
        def gen_tail(X):
            """Tail (combine + measure + output DMA) as a thunk list."""
            st = X.st
            scr1 = X.scr1
            acc = spool.tile([P, 2 * DIM], F16, tag="acc", name="acc")
            GG = pool.tile([P, NQ], F32, tag="GG", name="GG")
            cA = pool.tile([P, NQ], F32, tag="cA", name="cA")
            cB = pool.tile([P, NQ], F32, tag="cB", name="cB")
            hZ = pool.tile([P, NQ], F32, tag="hZ", name="hZ")
            scol = pool.tile([P, 8], F32, tag="scol", name="scol")
            msc32 = pool.tile([P, 2 * DIM], F32, tag="msc32", name="msc32")
            mscr = pool.tile([P, DIM], F16, tag="mscr", name="mscr")
            cfc = lambda k: cf_t[:, k : k + 1]

            def combine():
                w3 = pool.tile([P, 2 * DIM], F16, tag="w3", name="w3")
                w4 = pool.tile([P, 2 * DIM], F16, tag="w4", name="w4")
                if real_cf:
                    tsd(acc[:], st[0][:], cfc(0))
                    tsd(w3[:], st[1][:], cfc(3))
                    tsd(w4[:], st[2][:], cfc(6))
                    ttp(acc[:], acc[:], w3[:], op=AOT.add)
                    ttd(acc[:], acc[:], w4[:], op=AOT.add)
                else:
                    for pl in range(2):
                        out_sl = acc[:, pl * DIM : (pl + 1) * DIM]
                        for k in range(3):
                            s_re = cfc(3 * k) if pl == 0 else cfc(3 * k + 1)
                            s_im = cfc(3 * k + 2) if pl == 0 else cfc(3 * k)
                            dst = out_sl if k == 0 else scr1[:]
                            ax(dst, st[k][:, 0:DIM], s_re,
                               st[k][:, DIM : 2 * DIM], s_im)
                            if k > 0:
                                ttd(out_sl, out_sl, scr1[:], op=AOT.add)
                nc.scalar.activation(msc32[:], acc[:], ACTF.Square, accum_out=scol[:, 0:1])

            def _wire_views(plane, w):
                inner = 1 << (NQ - 1 - w)
                outer = HD // inner
                v = plane.rearrange("p (o t i) -> p o t i", o=outer, t=2, i=inner)
                return v[:, :, 0, :], v[:, :, 1, :]

            def meas_wire(w):
                inner = 1 << (NQ - 1 - w)
                outer = HD // inner
                accr, acci = acc[:, 0:DIM], acc[:, DIM : 2 * DIM]
                fv = acc[:].rearrange(
                    "p (m t i) -> p m t i", m=2 * outer, t=2, i=inner
                )
                p0b = fv[:, :, 0, :]
                p1b = fv[:, :, 1, :]
                ms2 = mscr[:].rearrange("p (m i) -> p m i", m=2 * outer, i=inner)
                ttd(ms2, p0b, p1b, op=AOT.add)
                nc.scalar.activation(
                    msc32[:, 0:DIM], mscr[:], ACTF.Square,
                    accum_out=GG[:, w : w + 1],
                )
                nc.scalar.activation(
                    msc32[:, DIM : 2 * DIM].rearrange(
                        "p (m i) -> p m i", m=2 * outer, i=inner
                    ),
                    p1b, ACTF.Square, accum_out=hZ[:, w : w + 1],
                )
                p0r, p1r = _wire_views(accr, w)
                p0i, p1i = _wire_views(acci, w)
                ms1 = mscr[:, 0:HD].rearrange("p (o i) -> p o i", o=outer, i=inner)
                nc.vector.scalar_tensor_tensor(
                    ms1, p0r, 0.0, p1i, op0=AOT.bypass, op1=AOT.mult,
                    accum_out=cA[:, w : w + 1],
                )
                nc.vector.scalar_tensor_tensor(
                    ms1, p0i, 0.0, p1r, op0=AOT.bypass, op1=AOT.mult,
                    accum_out=cB[:, w : w + 1],
                )

            def finalize():
                nc.vector.tensor_scalar(
                    scol[:, 1:2], scol[:, 0:1], 1e-9, None, op0=AOT.add
                )
                nc.vector.reciprocal(scol[:, 2:3], scol[:, 1:2])
                nc.vector.tensor_scalar(scol[:, 3:4], scol[:, 2:3], 2.0, None, op0=AOT.mult)
                nc.vector.tensor_scalar(scol[:, 4:5], scol[:, 2:3], -2.0, None, op0=AOT.mult)
                ttd(scol[:, 5:6], scol[:, 0:1], scol[:, 2:3], op=AOT.mult)
                nc.vector.tensor_scalar(scol[:, 6:7], scol[:, 5:6], -1.0, None, op0=AOT.mult)
                out30 = pool.tile([P, 30], F32, tag="out30", name="out30")
                wv_ = pool.tile([P, 10], F32, tag="wv", name="wv")
                nszb = scol[:, 6:7].broadcast_to([P, 1, NQ])
                nc.vector.scalar_tensor_tensor(
                    out30[:, 0:10].unsqueeze(1), GG[:].unsqueeze(1), scol[:, 2:3], nszb,
                    op0=AOT.mult, op1=AOT.add,
                )
                ttd(wv_[:], cA[:], cB[:], op=AOT.subtract)
                tsd(out30[:, 10:20], wv_[:], scol[:, 3:4])
                szb = scol[:, 5:6].broadcast_to([P, 1, NQ])
                nc.vector.scalar_tensor_tensor(
                    out30[:, 20:30].unsqueeze(1), hZ[:].unsqueeze(1), scol[:, 4:5], szb,
                    op0=AOT.mult, op1=AOT.add,
                )
                nc.sync.dma_start(out_d[X.r0:X.r1, :], out30[:])

            thunks = [combine]
            for w in range(NQ):
                thunks.append(lambda w=w: meas_wire(w))
            thunks.append(finalize)
            return thunks

        # ================= thunk generators =================
        def gen_rings(X, L):
            rings = [_ring_gates(c, L) for c in range(3)]
            thunks = []
            for k in range(2 * NQ):
                for c in range(3):
                    ctrl, tgt, col = rings[c][k]
                    idx = 3 * k + c   # deterministic per ring slot
                    form = CRX_PATTERN[idx % len(CRX_PATTERN)]
                    def th(X=X, c=c, ctrl=ctrl, tgt=tgt, col=col, form=form):
                        emit_crx(X, c, ctrl, tgt, col, form)
                    thunks.append(th)
            return thunks

        def gen_rot(X):
            thunks = []
            for w in range(NQ):
                for c in range(3):
                    thunks.append(lambda X=X, c=c, w=w: emit_rot(X, c, w))
            return thunks

        def weave(*lists):
            """Proportional round-robin emission of thunk lists."""
            lists = [l for l in lists if l]
            if not lists:
                return
            total = max(len(l) for l in lists)
            idx = [0] * len(lists)
            for step in range(total):
                for li, l in enumerate(lists):
                    want = (step + 1) * len(l) // total
                    while idx[li] < want:
                        l[idx[li]]()
                        idx[li] += 1

        # ================= pipelined schedule =================
        # phases per tile: P prologue, A rings0, B rot, C rings1, D tail.
        # Emission order (each phase exactly once, ~3 tiles in flight):
        #   P0 P1 P2 A0 [B0|A1] then per k:
        #     P(k+3), [C(k) | B(k+1) | A(k+2) | D(k-1)]
        # and D(n-1) at the end.
        n = n_tiles
        ctxs = {}
        ctxs[0] = emit_prologue(0)
        for th in ctxs[0].pro2:
            th()
        if n > 1:
            ctxs[1] = emit_prologue(1)
            for th in ctxs[1].pro2:
                th()
        weave(gen_rings(ctxs[0], 0))
        if n > 2:
            ctxs[2] = emit_prologue(2)
            for th in ctxs[2].pro2:
                th()
        weave(gen_rot(ctxs[0]), gen_rings(ctxs[1], 0) if n > 1 else [])
        for k in range(n):
            if k + 3 < n:
                ctxs[k + 3] = emit_prologue(k + 3)
                for th in ctxs[k + 3].pro2:
                    th()
            weave(
                gen_rings(ctxs[k], 1),
                gen_rot(ctxs[k + 1]) if k + 1 < n else [],
                gen_rings(ctxs[k + 2], 0) if k + 2 < n else [],
                gen_tail(ctxs[k - 1]) if k >= 1 else [],
            )
        weave(gen_tail(ctxs[n - 1]))


def build_nc(n_tiles=NT, b_core=None, real_cf=True):
    if b_core is None:
        b_core = n_tiles * P
    nc = bacc.Bacc("TRN2", target_bir_lowering=False)
    ins = {
        "ps": nc.dram_tensor("ps", [b_core, NCOL], F32, kind="ExternalInput")[:],
        "dth": nc.dram_tensor("dth", [b_core, 1], F32, kind="ExternalInput")[:],
        "cf": nc.dram_tensor("cf", [P, 9], F32, kind="ExternalInput")[:],
        "masks": nc.dram_tensor("masks", [P, 384], F16, kind="ExternalInput")[:],
    }
    outs = {"out": nc.dram_tensor("out", [b_core, 30], F32, kind="ExternalOutput")[:]}
    with tile.TileContext(nc) as tc:
        emit_core_kernel(nc, tc, ins, outs, n_tiles=n_tiles, real_cf=real_cf)
    nc.compile()
    return nc


_NC_CACHE = {}


def _get_nc(n_tiles=NT, real_cf=True):
    key = (n_tiles, real_cf)
    if key not in _NC_CACHE:
        _NC_CACHE[key] = build_nc(n_tiles, real_cf=real_cf)
    return _NC_CACHE[key]


def make_host_inputs(input_angles, forward_params, backward_params, diagonal_params,
                     dt_scale, alpha_real, alpha_imag, beta_real, beta_imag,
                     gamma_real, gamma_imag):
    """Host-side scalar prep + param column rearrangement."""
    al = complex(float(alpha_real), float(alpha_imag))
    be = complex(float(beta_real), float(beta_imag))
    ga = complex(float(gamma_real), float(gamma_imag))
    n = np.sqrt(abs(al) ** 2 + abs(be) ** 2 + abs(ga) ** 2 + 1e-9)
    cs = [al / n, be / n, ga / n]
    row = []
    for ck in cs:
        row += [ck.real, ck.imag, -ck.imag]
    cf = np.tile(np.asarray(row, np.float32), (P, 1))
    dth = (0.25 * np.asarray(dt_scale, np.float32)).reshape(-1, 1)

    B = np.asarray(input_angles).shape[0]
    ps = np.empty((B, NCOL), np.float32)
    pars = [np.asarray(forward_params, np.float32),
            np.asarray(backward_params, np.float32),
            np.asarray(diagonal_params, np.float32)]
    for c in range(3):
        par = pars[c]
        for L in range(2):
            base = 50 * L
            for w in range(NQ):
                g = w if c != BWD else NQ - 1 - w
                ps[:, rot_col(c, L, w, 0) : rot_col(c, L, w, 0) + 3] = (
                    par[:, base + 3 * g : base + 3 * g + 3]
                )
            ps[:, crx_col(c, L, 0) : crx_col(c, L, 0) + 20] = (
                par[:, base + 30 : base + 50]
            )
    ps[:, ANG0:ANG0 + NQ] = np.asarray(input_angles, np.float32)

    eye = np.eye(P, dtype=np.float16)
    masks = np.concatenate([eye, eye, -eye], axis=1)
    return cf, dth, ps, masks


def kernel(**inputs):
    from concourse.bass_utils import run_bass_kernel_spmd

    cf, dth, ps, masks = make_host_inputs(**inputs)
    real_cf = (
        abs(float(inputs["alpha_imag"])) < 1e-30
        and abs(float(inputs["beta_imag"])) < 1e-30
        and abs(float(inputs["gamma_imag"])) < 1e-30
    )
    nc = _get_nc(NT, real_cf)
    in_maps = []
    for c in range(N_CORES):
        r0, r1 = c * B_CORE, (c + 1) * B_CORE
        in_maps.append({
            "ps": np.ascontiguousarray(ps[r0:r1]),
            "dth": np.ascontiguousarray(dth[r0:r1]),
            "cf": cf,
            "masks": masks,
        })
    res = run_bass_kernel_spmd(nc, in_maps, core_ids=list(range(N_CORES)))
    out = np.concatenate([res.results[c]["out"] for c in range(N_CORES)], axis=0)
    return out.astype(np.float32)
